# revision 14
# baseline (speedup 1.0000x reference)
"""Trainium2 Bass kernel for the k-mer transformer problem (PE-matmul version).

Semantics (k=3, one-hot 3-mer filters over 4 bases):
    z[b, c, l] = relu(x[b,0,l,d0] + x[b,0,l+1,d1] + x[b,0,l+2,d2] - 2)
      where c = 16*d0 + 4*d1 + d2,  l in [0, 99999)
    out[b, 0, r*33333 + q, c] = z[b, c, 3q + r]      (mod-3 interleave)

Strategy: pure data parallel (batch elem b -> NeuronCore b). The channel
expansion (12 inputs -> 64 sums per position) runs on the PE array as a
matmul with a one-hot-sum stationary weight; the PSUM->SBUF relu drain is
split across ACT and DVE; the output is stored as uint8 (x255) so the
HBM store stream (6.4 MB, ~18 us) stays far below the drain wall.

  - for phase r (= l mod 3), position q reads x.flat[12q + 4r + j],
    j in [0,12). Positions are processed in PAIRS q = 2m+h, so the
    stationary is a [32, 128] block matrix S_r[4r + 12h + j, 64h + c] =
    Wk[j, c]; the phase shift 4r lives entirely in the stationary and
    PSUM partition p = 64h + c.
  - K=32 << 128, so the array runs in 4x ROW-TILED mode: m-chunks of 512
    are assigned round-robin to the four 32-row tiles (chunk j -> tile
    j%4 at tile_position (32*(j%4), 0)); the host stages the input
    QUARTERED: xt[32g + jj, 512t + u] = x.flat[24*(512*(4t+g) + u) + jj]
    ([128, 4608] fp16, 1.2 MB). Four MMs run concurrently in the array
    (~427 ns each at the cold 1.2 GHz HAM clock -> ~110 ns effective).
  - drains: chunks 4p,4p+1 -> ACT-dedicated PSUM pool, 4p+2,4p+3 -> DVE
    pool (2 bufs x 2 banks each = all 8 PSUM banks). Dedicated pools
    decouple the engines (v2's shared pool convoyed them). ACT computes
    Relu(255*z - 510) -> uint8; DVE computes (z-2)*255 -> uint8 (the
    saturating f32->u8 convert clamps negatives, doing the relu for
    free). Both write halves of one shared [128, 2048] u8 tile, so one
    store covers both (27 stores total -- v2's 54 stores at ~700 ns
    descriptor-gen each had the Sync sequencer near saturation).
  - input loaded in 4 pieces, all on the Sync HWDGE ring FIFO behind the
    raw-prefetched weights + piece 0 (v2 put pieces on the ACT ring,
    where they round-robined against the prefetch on the SDMA engines
    and pushed the first MM out by ~3 us).
  - host gather: y u8 [3, 128(=64h+c), 16672] -> transpose to [q, c],
    scale by 1/255. Total error ~2e-3 vs the 2e-2 gate (fp16 input
    staging ~1.5e-3 + u8 quantization ~4e-3 worst-case corners).
"""

import sys

import numpy as np

sys.path.insert(0, "/opt/trn_rl_repo")

import concourse.bacc as bacc  # noqa: E402
import concourse.mybir as mybir  # noqa: E402
from concourse.bass_utils import run_bass_kernel_spmd  # noqa: E402
from concourse.tile import TileContext  # noqa: E402

L = 100001
Q = 33333  # valid positions per phase (99999 / 3)
CH = 512  # m-chunk width (one matmul / one PSUM bank)
NCHUNK = 33  # chunks per phase; chunk 32 is 288 wide
TAILW = 288
H = 16672  # stored m-width per phase (= 32*512 + 288)
H4 = 4608  # staged cols per row-quarter (= 512 * ceil(33/4))
N_CORES = 8
# piece boundaries in staged xt columns; chunk j needs col 512*(j//4)
PIECE_EDGES = [0, 512, 1536, 3072, H4]

_CACHE = {}


def _build_bass():
    nc = bacc.Bacc()
    f32 = mybir.dt.float32
    f16 = mybir.dt.float16
    u8 = mybir.dt.uint8
    add = mybir.AluOpType.add
    mult = mybir.AluOpType.mult
    relu = mybir.ActivationFunctionType.Relu

    x_d = nc.declare_dram_parameter("x", [128, H4], f16, isOutput=False)
    w_d = nc.declare_dram_parameter("w", [128, 384], f16, isOutput=False)
    y_d = nc.declare_dram_parameter("y", [3, 128, H], u8, isOutput=True)

    # weights + piece 0 prefetched RAW, before the TileContext entry
    # barrier: Sync issues them right after iram+sem-init. The wait_ge is
    # emitted outside the Tile capture and precedes all tile-scheduled PE
    # work in the engine stream.
    p0w = PIECE_EDGES[1]
    w0 = nc.alloc_sbuf_tensor("wraw", [128, 384], f16)
    x0 = nc.alloc_sbuf_tensor("xraw0", [128, p0w], f16)
    pre = nc.alloc_semaphore("xpre0")
    nc.sync.dma_start(out=w0.ap(), in_=w_d[:, :]).then_inc(pre, 16)
    nc.sync.dma_start(out=x0.ap(), in_=x_d[:, 0:p0w]).then_inc(pre, 16)
    nc.tensor.wait_ge(pre, 32)

    n_pieces = len(PIECE_EDGES) - 1

    def piece_of(col):
        for i in range(n_pieces):
            if col < PIECE_EDGES[i + 1]:
                return i
        raise AssertionError(col)

    with TileContext(nc) as tc:
        with (
            tc.tile_pool(name="xp", bufs=1) as xp,
            tc.tile_pool(name="psa", bufs=2, space="PSUM") as psa,
            tc.tile_pool(name="psd", bufs=2, space="PSUM") as psd,
            tc.tile_pool(name="op_", bufs=4) as op_,
        ):
            bias_a = xp.tile([128, 1], f32, tag="bias")
            nc.vector.memset(bias_a, -510.0)
            # pieces 1..n on the Sync ring: FIFO behind the raw prefetch,
            # ahead of all stores
            px = {0: x0.ap()}
            for i in range(1, n_pieces):
                c0, c1 = PIECE_EDGES[i], PIECE_EDGES[i + 1]
                t = xp.tile([128, c1 - c0], f16, tag=f"px{i}")
                nc.sync.dma_start(out=t, in_=x_d[:, c0:c1])
                px[i] = t

            def mm(j, pt, half):
                """Matmul chunk j (phase-r stationary wr) into psum half."""
                g = j % 4
                col = CH * (j // 4)
                cw = TAILW if j == NCHUNK - 1 else CH
                pi = piece_of(col)
                pb = PIECE_EDGES[pi]
                nc.tensor.matmul(
                    pt[:, CH * half : CH * half + cw],
                    w0.ap()[32 * g : 32 * g + 32, 128 * r : 128 * r + 128],
                    px[pi][32 * g : 32 * g + 32, col - pb : col - pb + cw],
                    start=True,
                    stop=True,
                    tile_position=(32 * g, 0),
                )

            def drain_a(pt, w, o_ap):
                # ACT: Relu(255*z - 510) = 255*relu(z-2)
                nc.scalar.activation(
                    o_ap, pt[:, :w], relu, bias=bias_a, scale=255.0
                )

            def drain_d(pt, w, o_ap):
                # DVE: (z-2)*255; saturating u8 convert clamps to [0,255]
                nc.vector.tensor_scalar(o_ap, pt[:, :w], -2.0, 255.0, add, mult)

            def tail():
                """Ragged chunk 32 (288 wide) on ACT."""
                pa = psa.tile([128, 2 * CH], f32, tag="psa")
                mm(NCHUNK - 1, pa, 0)
                ot = op_.tile([128, TAILW], u8, tag="ot")
                drain_a(pa, TAILW, ot)
                nc.sync.dma_start(out=y_d[r, :, H - TAILW : H], in_=ot)

            def group(j0, split_store=False):
                """Four chunks: psa pair on ACT + psd pair on DVE."""
                pa = psa.tile([128, 2 * CH], f32, tag="psa")
                pd = psd.tile([128, 2 * CH], f32, tag="psd")
                mm(j0 + 0, pa, 0)
                mm(j0 + 1, pa, 1)
                mm(j0 + 2, pd, 0)
                mm(j0 + 3, pd, 1)
                m0 = CH * j0
                if split_store:
                    # final group: each half ships as soon as its drain
                    # lands, so the kernel's last store is small and early
                    oa = op_.tile([128, 2 * CH], u8, tag="oh")
                    drain_a(pa, 2 * CH, oa)
                    nc.sync.dma_start(out=y_d[r, :, m0 : m0 + 2 * CH], in_=oa)
                    od = op_.tile([128, 2 * CH], u8, tag="oh")
                    drain_d(pd, 2 * CH, od)
                    nc.sync.dma_start(
                        out=y_d[r, :, m0 + 2 * CH : m0 + 4 * CH], in_=od
                    )
                else:
                    o = op_.tile([128, 4 * CH], u8, tag="o")
                    drain_a(pa, 2 * CH, o[:, 0 : 2 * CH])
                    drain_d(pd, 2 * CH, o[:, 2 * CH : 4 * CH])
                    nc.sync.dma_start(out=y_d[r, :, m0 : m0 + 4 * CH], in_=o)

            # the ragged tail goes early (it needs piece 3, so phase 0
            # runs it late but not last) -- the kernel's final store is
            # then a regular pipelined pair store, not a small exposed
            # one on the critical path.
            for r in range(3):
                if r > 0:
                    tail()
                for p in range(8):
                    group(4 * p, split_store=(r == 2 and p == 7))
                    if r == 0 and p == 6:
                        tail()
    return nc


def _kmer_w():
    """Stationary [128, 384] fp16: rows [32g,32g+32) all hold S, cols
    [128r, 128r+128) = S_r[jj, 64h+c] = Wk[jj-4r-12h, c]."""
    c = np.arange(64)
    digits = np.stack([c // 16, (c // 4) % 4, c % 4])  # [t, c]
    wk = np.zeros((12, 64), np.float32)
    for t in range(3):
        for d in range(4):
            wk[4 * t + d] = digits[t] == d
    w = np.zeros((128, 384), np.float32)
    for g in range(4):
        for r in range(3):
            for h in range(2):
                w[
                    32 * g + 4 * r + 12 * h : 32 * g + 4 * r + 12 * h + 12,
                    128 * r + 64 * h : 128 * r + 64 * h + 64,
                ] = wk
    return w.astype(np.float16)


def _stage_inputs(x):
    """x: [8,1,L,4] f32 -> per-core {'x': [128, H4] f16, 'w': [128,384] f16}.

    xt[32g + jj, 512t + u] = x.flat[24*(512*(4t+g) + u) + jj]  (0 padded).
    """
    w = _kmer_w()
    need = 24 * (CH * (4 * 8 + 3) + CH - 1) + 32  # max strided index + 1
    in_maps = []
    for b in range(x.shape[0]):
        xf = np.zeros(need, dtype=np.float16)
        xf[: L * 4] = x[b, 0].ravel().astype(np.float16)
        xt = np.empty((128, H4), dtype=np.float16)
        for g in range(4):
            v = np.lib.stride_tricks.as_strided(
                xf[24 * CH * g :],
                shape=(32, 9, CH),
                strides=(2, 24 * 4 * CH * 2, 48),
            )
            xt[32 * g : 32 * g + 32] = v.reshape(32, H4)
        in_maps.append({"x": xt, "w": w})
    return in_maps


def _gather_output(results):
    out = np.empty((len(results), 1, 3 * Q, 64), dtype=np.float32)
    scale = np.float32(1.0 / 255.0)
    for b, res in enumerate(results):
        y = res["y"].reshape(3, 2, 64, H)  # [r, h, c, m] uint8
        y = y.transpose(0, 3, 1, 2).reshape(3, 2 * H, 64)[:, :Q, :]
        out[b, 0] = y.reshape(3 * Q, 64).astype(np.float32) * scale
    return out


def _built_and_finalized():
    if "nc" not in _CACHE:
        nc = _build_bass()
        nc.finalize()
        _CACHE["nc"] = nc
    return _CACHE["nc"]


def run(x, trace=False):
    nc = _built_and_finalized()
    in_maps = _stage_inputs(np.asarray(x, dtype=np.float32))
    bkr = run_bass_kernel_spmd(nc, in_maps, list(range(N_CORES)), trace=trace)
    return _gather_output(bkr.results), bkr


def kernel(x, W=None):
    out, _ = run(x, trace=False)
    return out


# revision 15
# speedup vs baseline: 1.0616x; 1.0616x over previous
"""Trainium2 Bass kernel for the k-mer transformer problem (PE-matmul version).

Semantics (k=3, one-hot 3-mer filters over 4 bases):
    z[b, c, l] = relu(x[b,0,l,d0] + x[b,0,l+1,d1] + x[b,0,l+2,d2] - 2)
      where c = 16*d0 + 4*d1 + d2,  l in [0, 99999)
    out[b, 0, r*33333 + q, c] = z[b, c, 3q + r]      (mod-3 interleave)

Strategy: pure data parallel (batch elem b -> NeuronCore b). The channel
expansion (12 inputs -> 64 sums per position) runs on the PE array as a
matmul with a one-hot-sum stationary weight; the PSUM->SBUF relu drain is
split across ACT and DVE; the output is stored as uint8 (x255) so the
HBM store stream (6.4 MB, ~18 us) stays far below the drain wall.

  - for phase r (= l mod 3), position q reads x.flat[12q + 4r + j],
    j in [0,12). Positions are processed in PAIRS q = 2m+h, so the
    stationary is a [32, 128] block matrix S_r[4r + 12h + j, 64h + c] =
    Wk[j, c]; the phase shift 4r lives entirely in the stationary and
    PSUM partition p = 64h + c.
  - K=32 << 128, so the array runs in 4x ROW-TILED mode: m-chunks of 512
    are assigned round-robin to the four 32-row tiles (chunk j -> tile
    j%4 at tile_position (32*(j%4), 0)); the host stages the input
    QUARTERED: xt[32g + jj, 512t + u] = x.flat[24*(512*(4t+g) + u) + jj]
    ([128, 4608] fp16, 1.2 MB). Four MMs run concurrently in the array
    (~427 ns each at the cold 1.2 GHz HAM clock -> ~110 ns effective).
  - drains: chunks 4p,4p+1 -> ACT-dedicated PSUM pool, 4p+2,4p+3 -> DVE
    pool (2 bufs x 2 banks each = all 8 PSUM banks). Dedicated pools
    decouple the engines (v2's shared pool convoyed them). ACT computes
    Relu(255*z - 510) -> uint8; DVE computes (z-2)*255 -> uint8 (the
    saturating f32->u8 convert clamps negatives, doing the relu for
    free). Both write halves of one shared [128, 2048] u8 tile, so one
    store covers both (27 stores total -- v2's 54 stores at ~700 ns
    descriptor-gen each had the Sync sequencer near saturation).
  - input loaded in 4 pieces, all on the Sync HWDGE ring FIFO behind the
    raw-prefetched weights + piece 0 (v2 put pieces on the ACT ring,
    where they round-robined against the prefetch on the SDMA engines
    and pushed the first MM out by ~3 us).
  - host gather: y u8 [3, 128(=64h+c), 16672] -> transpose to [q, c],
    scale by 1/255. Total error ~2e-3 vs the 2e-2 gate (fp16 input
    staging ~1.5e-3 + u8 quantization ~4e-3 worst-case corners).
"""

import sys

import numpy as np

sys.path.insert(0, "/opt/trn_rl_repo")

import concourse.bacc as bacc  # noqa: E402
import concourse.mybir as mybir  # noqa: E402
from concourse.bass_utils import run_bass_kernel_spmd  # noqa: E402
from concourse.tile import TileContext  # noqa: E402

L = 100001
Q = 33333  # valid positions per phase (99999 / 3)
CH = 512  # m-chunk width (one matmul / one PSUM bank)
NCHUNK = 33  # chunks per phase; chunk 32 is 288 wide
TAILW = 288
H = 16672  # stored m-width per phase (= 32*512 + 288)
H4 = 4608  # staged cols per row-quarter (= 512 * ceil(33/4))
N_CORES = 8
# piece boundaries in staged xt columns; chunk j needs col 512*(j//4)
PIECE_EDGES = [0, 512, 1536, 3072, H4]

_CACHE = {}


def _build_bass():
    nc = bacc.Bacc()
    f32 = mybir.dt.float32
    f16 = mybir.dt.float16
    u8 = mybir.dt.uint8
    add = mybir.AluOpType.add
    mult = mybir.AluOpType.mult
    relu = mybir.ActivationFunctionType.Relu

    x_d = nc.declare_dram_parameter("x", [128, H4], f16, isOutput=False)
    w_d = nc.declare_dram_parameter("w", [128, 384], f16, isOutput=False)
    y_d = nc.declare_dram_parameter("y", [3, 128, H], u8, isOutput=True)

    # weights + piece 0 prefetched RAW, before the TileContext entry
    # barrier: Sync issues them right after iram+sem-init. The wait_ge is
    # emitted outside the Tile capture and precedes all tile-scheduled PE
    # work in the engine stream.
    p0w = PIECE_EDGES[1]
    w0 = nc.alloc_sbuf_tensor("wraw", [128, 384], f16)
    x0 = nc.alloc_sbuf_tensor("xraw0", [128, p0w], f16)
    pre = nc.alloc_semaphore("xpre0")
    nc.sync.dma_start(out=w0.ap(), in_=w_d[:, :]).then_inc(pre, 16)
    nc.sync.dma_start(out=x0.ap(), in_=x_d[:, 0:p0w]).then_inc(pre, 16)
    nc.tensor.wait_ge(pre, 32)

    n_pieces = len(PIECE_EDGES) - 1

    def piece_of(col):
        for i in range(n_pieces):
            if col < PIECE_EDGES[i + 1]:
                return i
        raise AssertionError(col)

    with TileContext(nc) as tc:
        with (
            tc.tile_pool(name="xp", bufs=1) as xp,
            tc.tile_pool(name="psa", bufs=2, space="PSUM") as psa,
            tc.tile_pool(name="psd", bufs=2, space="PSUM") as psd,
            tc.tile_pool(name="op_", bufs=4) as op_,
        ):
            bias_a = xp.tile([128, 1], f32, tag="bias")
            nc.vector.memset(bias_a, -510.0)
            # pieces 1..n on the Sync ring: FIFO behind the raw prefetch,
            # ahead of all stores
            px = {0: x0.ap()}
            for i in range(1, n_pieces):
                c0, c1 = PIECE_EDGES[i], PIECE_EDGES[i + 1]
                t = xp.tile([128, c1 - c0], f16, tag=f"px{i}")
                nc.sync.dma_start(out=t, in_=x_d[:, c0:c1])
                px[i] = t

            def mm(j, pt, half):
                """Matmul chunk j (phase-r stationary wr) into psum half."""
                g = j % 4
                col = CH * (j // 4)
                cw = TAILW if j == NCHUNK - 1 else CH
                pi = piece_of(col)
                pb = PIECE_EDGES[pi]
                nc.tensor.matmul(
                    pt[:, CH * half : CH * half + cw],
                    w0.ap()[32 * g : 32 * g + 32, 128 * r : 128 * r + 128],
                    px[pi][32 * g : 32 * g + 32, col - pb : col - pb + cw],
                    start=True,
                    stop=True,
                    tile_position=(32 * g, 0),
                )

            def drain_a(pt, w, o_ap):
                # ACT: Relu(255*z - 510) = 255*relu(z-2)
                nc.scalar.activation(
                    o_ap, pt[:, :w], relu, bias=bias_a, scale=255.0
                )

            def drain_d(pt, w, o_ap):
                # DVE: (z-2)*255; saturating u8 convert clamps to [0,255]
                nc.vector.tensor_scalar(o_ap, pt[:, :w], -2.0, 255.0, add, mult)

            def tail():
                """Ragged chunk 32 (288 wide) on ACT."""
                pa = psa.tile([128, 2 * CH], f32, tag="psa")
                mm(NCHUNK - 1, pa, 0)
                ot = op_.tile([128, TAILW], u8, tag="ot")
                drain_a(pa, TAILW, ot)
                nc.sync.dma_start(out=y_d[r, :, H - TAILW : H], in_=ot)

            def group(j0):
                """Four chunks: psa pair on ACT + psd pair on DVE."""
                pa = psa.tile([128, 2 * CH], f32, tag="psa")
                pd = psd.tile([128, 2 * CH], f32, tag="psd")
                mm(j0 + 0, pa, 0)
                mm(j0 + 1, pa, 1)
                mm(j0 + 2, pd, 0)
                mm(j0 + 3, pd, 1)
                o = op_.tile([128, 4 * CH], u8, tag="o")
                drain_a(pa, 2 * CH, o[:, 0 : 2 * CH])
                drain_d(pd, 2 * CH, o[:, 2 * CH : 4 * CH])
                m0 = CH * j0
                nc.sync.dma_start(out=y_d[r, :, m0 : m0 + 4 * CH], in_=o)

            # the ragged tail goes early (it needs piece 3, so phase 0
            # runs it late but not last) -- the kernel's final store is
            # then a regular pipelined pair store, not a small exposed
            # one on the critical path.
            for r in range(3):
                if r > 0:
                    tail()
                for p in range(8):
                    group(4 * p)
                    if r == 0 and p == 6:
                        tail()
    return nc


def _kmer_w():
    """Stationary [128, 384] fp16: rows [32g,32g+32) all hold S, cols
    [128r, 128r+128) = S_r[jj, 64h+c] = Wk[jj-4r-12h, c]."""
    c = np.arange(64)
    digits = np.stack([c // 16, (c // 4) % 4, c % 4])  # [t, c]
    wk = np.zeros((12, 64), np.float32)
    for t in range(3):
        for d in range(4):
            wk[4 * t + d] = digits[t] == d
    w = np.zeros((128, 384), np.float32)
    for g in range(4):
        for r in range(3):
            for h in range(2):
                w[
                    32 * g + 4 * r + 12 * h : 32 * g + 4 * r + 12 * h + 12,
                    128 * r + 64 * h : 128 * r + 64 * h + 64,
                ] = wk
    return w.astype(np.float16)


def _stage_inputs(x):
    """x: [8,1,L,4] f32 -> per-core {'x': [128, H4] f16, 'w': [128,384] f16}.

    xt[32g + jj, 512t + u] = x.flat[24*(512*(4t+g) + u) + jj]  (0 padded).
    """
    w = _kmer_w()
    need = 24 * (CH * (4 * 8 + 3) + CH - 1) + 32  # max strided index + 1
    in_maps = []
    for b in range(x.shape[0]):
        xf = np.zeros(need, dtype=np.float16)
        xf[: L * 4] = x[b, 0].ravel().astype(np.float16)
        xt = np.empty((128, H4), dtype=np.float16)
        for g in range(4):
            v = np.lib.stride_tricks.as_strided(
                xf[24 * CH * g :],
                shape=(32, 9, CH),
                strides=(2, 24 * 4 * CH * 2, 48),
            )
            xt[32 * g : 32 * g + 32] = v.reshape(32, H4)
        in_maps.append({"x": xt, "w": w})
    return in_maps


def _gather_output(results):
    out = np.empty((len(results), 1, 3 * Q, 64), dtype=np.float32)
    scale = np.float32(1.0 / 255.0)
    for b, res in enumerate(results):
        y = res["y"].reshape(3, 2, 64, H)  # [r, h, c, m] uint8
        y = y.transpose(0, 3, 1, 2).reshape(3, 2 * H, 64)[:, :Q, :]
        out[b, 0] = y.reshape(3 * Q, 64).astype(np.float32) * scale
    return out


def _built_and_finalized():
    if "nc" not in _CACHE:
        nc = _build_bass()
        nc.finalize()
        _CACHE["nc"] = nc
    return _CACHE["nc"]


def run(x, trace=False):
    nc = _built_and_finalized()
    in_maps = _stage_inputs(np.asarray(x, dtype=np.float32))
    bkr = run_bass_kernel_spmd(nc, in_maps, list(range(N_CORES)), trace=trace)
    return _gather_output(bkr.results), bkr


def kernel(x, W=None):
    out, _ = run(x, trace=False)
    return out


# revision 16
# speedup vs baseline: 1.0732x; 1.0110x over previous
"""Trainium2 Bass kernel for the k-mer transformer problem (PE-matmul version).

Semantics (k=3, one-hot 3-mer filters over 4 bases):
    z[b, c, l] = relu(x[b,0,l,d0] + x[b,0,l+1,d1] + x[b,0,l+2,d2] - 2)
      where c = 16*d0 + 4*d1 + d2,  l in [0, 99999)
    out[b, 0, r*33333 + q, c] = z[b, c, 3q + r]      (mod-3 interleave)

Strategy: pure data parallel (batch elem b -> NeuronCore b). The channel
expansion (12 inputs -> 64 sums per position) runs on the PE array as a
matmul with a one-hot-sum stationary weight; the PSUM->SBUF relu drain is
split across ACT and DVE; the output is stored as uint8 (x255) so the
HBM store stream (6.4 MB, ~18 us) stays far below the drain wall.

  - for phase r (= l mod 3), position q reads x.flat[12q + 4r + j],
    j in [0,12). Positions are processed in PAIRS q = 2m+h, so the
    stationary is a [32, 128] block matrix S_r[4r + 12h + j, 64h + c] =
    Wk[j, c]; the phase shift 4r lives entirely in the stationary and
    PSUM partition p = 64h + c.
  - K=32 << 128, so the array runs in 4x ROW-TILED mode: m-chunks of 512
    are assigned round-robin to the four 32-row tiles (chunk j -> tile
    j%4 at tile_position (32*(j%4), 0)); the host stages the input
    QUARTERED: xt[32g + jj, 512t + u] = x.flat[24*(512*(4t+g) + u) + jj]
    ([128, 4608] fp16, 1.2 MB). Four MMs run concurrently in the array
    (~427 ns each at the cold 1.2 GHz HAM clock -> ~110 ns effective).
  - drains: chunks 4p,4p+1 -> ACT-dedicated PSUM pool, 4p+2,4p+3 -> DVE
    pool (2 bufs x 2 banks each = all 8 PSUM banks). Dedicated pools
    decouple the engines (v2's shared pool convoyed them). ACT computes
    Relu(255*z - 510) -> uint8; DVE computes (z-2)*255 -> uint8 (the
    saturating f32->u8 convert clamps negatives, doing the relu for
    free). Both write halves of one shared [128, 2048] u8 tile, so one
    store covers both (27 stores total -- v2's 54 stores at ~700 ns
    descriptor-gen each had the Sync sequencer near saturation).
  - input loaded in 4 pieces, all on the Sync HWDGE ring FIFO behind the
    raw-prefetched weights + piece 0 (v2 put pieces on the ACT ring,
    where they round-robined against the prefetch on the SDMA engines
    and pushed the first MM out by ~3 us).
  - host gather: y u8 [3, 128(=64h+c), 16672] -> transpose to [q, c],
    scale by 1/255. Total error ~2e-3 vs the 2e-2 gate (fp16 input
    staging ~1.5e-3 + u8 quantization ~4e-3 worst-case corners).
"""

import sys

import numpy as np

sys.path.insert(0, "/opt/trn_rl_repo")

import concourse.bacc as bacc  # noqa: E402
import concourse.mybir as mybir  # noqa: E402
from concourse.bass_utils import run_bass_kernel_spmd  # noqa: E402
from concourse.tile import TileContext  # noqa: E402

L = 100001
Q = 33333  # valid positions per phase (99999 / 3)
CH = 512  # m-chunk width (one matmul / one PSUM bank)
NCHUNK = 33  # chunks per phase; chunk 32 is 288 wide
TAILW = 288
H = 16672  # stored m-width per phase (= 32*512 + 288)
H4 = 4608  # staged cols per row-quarter (= 512 * ceil(33/4))
N_CORES = 8
# piece boundaries in staged xt columns; chunk j needs col 512*(j//4)
PIECE_EDGES = [0, 512, 1536, 3072, H4]

_CACHE = {}


def _build_bass():
    nc = bacc.Bacc()
    f32 = mybir.dt.float32
    f16 = mybir.dt.float16
    u8 = mybir.dt.uint8
    add = mybir.AluOpType.add
    mult = mybir.AluOpType.mult
    relu = mybir.ActivationFunctionType.Relu

    x_d = nc.declare_dram_parameter("x", [128, H4], f16, isOutput=False)
    w_d = nc.declare_dram_parameter("w", [128, 384], f16, isOutput=False)
    y_d = nc.declare_dram_parameter("y", [3, 128, H], u8, isOutput=True)

    # weights + piece 0 prefetched RAW, before the TileContext entry
    # barrier: Sync issues them right after iram+sem-init. The wait_ge is
    # emitted outside the Tile capture and precedes all tile-scheduled PE
    # work in the engine stream.
    p0w = PIECE_EDGES[1]
    w0 = nc.alloc_sbuf_tensor("wraw", [128, 384], f16)
    x0 = nc.alloc_sbuf_tensor("xraw0", [128, p0w], f16)
    pre = nc.alloc_semaphore("xpre0")
    nc.sync.dma_start(out=w0.ap(), in_=w_d[:, :]).then_inc(pre, 16)
    nc.sync.dma_start(out=x0.ap(), in_=x_d[:, 0:p0w]).then_inc(pre, 16)
    nc.tensor.wait_ge(pre, 32)

    n_pieces = len(PIECE_EDGES) - 1

    def piece_of(col):
        for i in range(n_pieces):
            if col < PIECE_EDGES[i + 1]:
                return i
        raise AssertionError(col)

    with TileContext(nc) as tc:
        with (
            tc.tile_pool(name="xp", bufs=1) as xp,
            tc.tile_pool(name="psa", bufs=2, space="PSUM") as psa,
            tc.tile_pool(name="psd", bufs=2, space="PSUM") as psd,
            tc.tile_pool(name="op_", bufs=6) as op_,
        ):
            bias_a = xp.tile([128, 1], f32, tag="bias")
            nc.vector.memset(bias_a, -510.0)
            # pieces 1..n on the Sync ring: FIFO behind the raw prefetch,
            # ahead of all stores
            px = {0: x0.ap()}
            for i in range(1, n_pieces):
                c0, c1 = PIECE_EDGES[i], PIECE_EDGES[i + 1]
                t = xp.tile([128, c1 - c0], f16, tag=f"px{i}")
                nc.sync.dma_start(out=t, in_=x_d[:, c0:c1])
                px[i] = t

            def mm(j, pt, half):
                """Matmul chunk j (phase-r stationary wr) into psum half."""
                g = j % 4
                col = CH * (j // 4)
                cw = TAILW if j == NCHUNK - 1 else CH
                pi = piece_of(col)
                pb = PIECE_EDGES[pi]
                nc.tensor.matmul(
                    pt[:, CH * half : CH * half + cw],
                    w0.ap()[32 * g : 32 * g + 32, 128 * r : 128 * r + 128],
                    px[pi][32 * g : 32 * g + 32, col - pb : col - pb + cw],
                    start=True,
                    stop=True,
                    tile_position=(32 * g, 0),
                )

            def drain_a(pt, w, o_ap):
                # ACT: Relu(255*z - 510) = 255*relu(z-2)
                nc.scalar.activation(
                    o_ap, pt[:, :w], relu, bias=bias_a, scale=255.0
                )

            def drain_d(pt, w, o_ap):
                # DVE: (z-2)*255; saturating u8 convert clamps to [0,255]
                nc.vector.tensor_scalar(o_ap, pt[:, :w], -2.0, 255.0, add, mult)

            def tail():
                """Ragged chunk 32 (288 wide) on ACT."""
                pa = psa.tile([128, 2 * CH], f32, tag="psa")
                mm(NCHUNK - 1, pa, 0)
                ot = op_.tile([128, TAILW], u8, tag="ot")
                drain_a(pa, TAILW, ot)
                nc.sync.dma_start(out=y_d[r, :, H - TAILW : H], in_=ot)

            def group(j0):
                """Four chunks: psa pair on ACT + psd pair on DVE."""
                pa = psa.tile([128, 2 * CH], f32, tag="psa")
                pd = psd.tile([128, 2 * CH], f32, tag="psd")
                mm(j0 + 0, pa, 0)
                mm(j0 + 1, pa, 1)
                mm(j0 + 2, pd, 0)
                mm(j0 + 3, pd, 1)
                o = op_.tile([128, 4 * CH], u8, tag="o")
                drain_a(pa, 2 * CH, o[:, 0 : 2 * CH])
                drain_d(pd, 2 * CH, o[:, 2 * CH : 4 * CH])
                m0 = CH * j0
                nc.sync.dma_start(out=y_d[r, :, m0 : m0 + 4 * CH], in_=o)

            # the ragged tail goes early (it needs piece 3, so phase 0
            # runs it late but not last) -- the kernel's final store is
            # then a regular pipelined pair store, not a small exposed
            # one on the critical path.
            for r in range(3):
                if r > 0:
                    tail()
                for p in range(8):
                    group(4 * p)
                    if r == 0 and p == 6:
                        tail()
    return nc


def _kmer_w():
    """Stationary [128, 384] fp16: rows [32g,32g+32) all hold S, cols
    [128r, 128r+128) = S_r[jj, 64h+c] = Wk[jj-4r-12h, c]."""
    c = np.arange(64)
    digits = np.stack([c // 16, (c // 4) % 4, c % 4])  # [t, c]
    wk = np.zeros((12, 64), np.float32)
    for t in range(3):
        for d in range(4):
            wk[4 * t + d] = digits[t] == d
    w = np.zeros((128, 384), np.float32)
    for g in range(4):
        for r in range(3):
            for h in range(2):
                w[
                    32 * g + 4 * r + 12 * h : 32 * g + 4 * r + 12 * h + 12,
                    128 * r + 64 * h : 128 * r + 64 * h + 64,
                ] = wk
    return w.astype(np.float16)


def _stage_inputs(x):
    """x: [8,1,L,4] f32 -> per-core {'x': [128, H4] f16, 'w': [128,384] f16}.

    xt[32g + jj, 512t + u] = x.flat[24*(512*(4t+g) + u) + jj]  (0 padded).
    """
    w = _kmer_w()
    need = 24 * (CH * (4 * 8 + 3) + CH - 1) + 32  # max strided index + 1
    in_maps = []
    for b in range(x.shape[0]):
        xf = np.zeros(need, dtype=np.float16)
        xf[: L * 4] = x[b, 0].ravel().astype(np.float16)
        xt = np.empty((128, H4), dtype=np.float16)
        for g in range(4):
            v = np.lib.stride_tricks.as_strided(
                xf[24 * CH * g :],
                shape=(32, 9, CH),
                strides=(2, 24 * 4 * CH * 2, 48),
            )
            xt[32 * g : 32 * g + 32] = v.reshape(32, H4)
        in_maps.append({"x": xt, "w": w})
    return in_maps


def _gather_output(results):
    out = np.empty((len(results), 1, 3 * Q, 64), dtype=np.float32)
    scale = np.float32(1.0 / 255.0)
    for b, res in enumerate(results):
        y = res["y"].reshape(3, 2, 64, H)  # [r, h, c, m] uint8
        y = y.transpose(0, 3, 1, 2).reshape(3, 2 * H, 64)[:, :Q, :]
        out[b, 0] = y.reshape(3 * Q, 64).astype(np.float32) * scale
    return out


def _built_and_finalized():
    if "nc" not in _CACHE:
        nc = _build_bass()
        nc.finalize()
        _CACHE["nc"] = nc
    return _CACHE["nc"]


def run(x, trace=False):
    nc = _built_and_finalized()
    in_maps = _stage_inputs(np.asarray(x, dtype=np.float32))
    bkr = run_bass_kernel_spmd(nc, in_maps, list(range(N_CORES)), trace=trace)
    return _gather_output(bkr.results), bkr


def kernel(x, W=None):
    out, _ = run(x, trace=False)
    return out
